# revision 40
# baseline (speedup 1.0000x reference)
"""Trainium2 Bass kernel for nn_MultiHeadedAttention_25984552141341.

Computation (reference):
    q/k/v = (x @ W + b) split into 8 heads of 64
    scores = q @ k^T / 8
    scores += sf_net(scores)   (SoftmaxResNet over the key dim, 71)
    p = softmax(scores, axis=key)
    out = (p @ v merged) @ Wo + bo

Sharding: batch (512) split across 8 NeuronCores, 64 batches each.
All weights replicated. Each core runs an identical Bass program (SPMD).

Device-side layout strategy (per core):
  * activations live feature-on-partition / token-on-free ("transposed"
    layout, xT = [D, B*L]); the host passes query/key/value pre-transposed
    so no on-chip transposes of the big inputs are needed.
  * qT, kT produced as [dout, tok] (transposed) by matmuls with the weight
    tiles as the stationary operand; v produced per-batch in natural
    [tok, dout] layout (needed as the moving operand of p @ v).
  * scores are computed transposed: S' = [k, q] = kT_h.T @ qT_h, which is
    exactly the layout the sf-net matmuls want (contraction over k).
  * sf-net gelu is evaluated with the tanh approximation so that every
    scalar-engine activation (Square, Tanh, Exp, Identity) lives in the
    single `exp_and_others` LUT set -- the baseline's per-pair
    gelu<->exp ACT_TABLE_LOADs (1.3us each, 64 total) vanish entirely.
      gelu(x) ~= 0.5 x (1 + tanh(0.79788(x + 0.044715 x^3)))
    evaluated as   x2 = Square(x)               [scalar]
                   u  = (x2 + 22.3639) * x      [gpsimd]
                   th = Tanh(0.0356774 * u)     [scalar]
                   g2 = (th + 1) * x = 2gelu(x) [gpsimd]
    with w2 pre-halved on device to absorb the factor 2, and b1 folded
    into the h1 matmul as an extra stationary row (row 71 = b1) against
    a constant ones-row appended to the moving scores tile.
  * h1/h2 run in two 284-wide halves, each exactly one PSUM bank and
    aligned with the scores' head-bank split, so the gelu chain and the
    h2 accumulation pipeline per half.
  * softmax without max-subtraction (|scores2| < ~3): exp(+b2) on scalar.
  * attention: one matmul per head with stationary E'_h = exp(scores2)
    and moving [v_h | 1] producing [q, dh] plus the softmax denominator;
    normalization by per-partition (per-q) reciprocal (vector) and the
    scale-multiply on gpsimd.
  * attn rows are transposed back per batch with the PE transpose and
    assembled into attnT [D, tok] feeding the output projection.
  * the output projection is emitted per pair-of-groups (1136 tokens)
    as PE "filler" work inside the next groups' stall windows; the
    next group's q/k projections are likewise emitted as fillers inside
    the current group's pair loop, keeping the PE fed while the
    scalar/gpsimd gelu+exp chain runs.
  * biases: bq/bk folded into the PSUM->SBUF copies, bv folded into the
    output bias (softmax rows sum to 1), bo_eff = bo + bv @ Wo computed
    on device once.

All matmuls run in bf16 (fp32 PSUM accumulation).
"""

import contextlib
from collections import deque

import numpy as np

import concourse.bass as bass
import concourse.mybir as mybir
import concourse.tile as tile
from concourse import bacc
from concourse import bass_utils
from concourse.masks import make_identity

F32 = mybir.dt.float32
BF16 = mybir.dt.bfloat16
AF = mybir.ActivationFunctionType
ALU = mybir.AluOpType

N_CORES = 8
B, L, D, H = 512, 71, 512, 8
DH = D // H  # 64
FF = 128  # sf_net hidden
BC = B // N_CORES  # 64 batches per core
T = BC * L  # 4544 tokens per core
GB = 8  # batches per group
G = BC // GB  # 8 groups
GT = GB * L  # 568 tokens per group
HALF = GT // 2  # 284
DUO = 2 * GT  # 1136 tokens per pair of groups

# tanh-gelu constants: gelu(x) = 0.5x(1+tanh(sqrt(2/pi)(x + c x^3)))
#   u = x(x^2 + 1/c); th = tanh(sqrt(2/pi)*c * u); 2*gelu = (th+1)x
GC1 = 22.363860002236386
GC2 = 0.035677408136300125

_CACHE = {}


def _build():
    nc = bacc.Bacc("TRN2", target_bir_lowering=False, debug=False,
                   num_devices=N_CORES)

    # big tensors arrive host-converted to bf16 (they were down-converted
    # to bf16 by the DMA anyway -- same numerics, half the HBM traffic)
    xqT = nc.dram_tensor("xqT", [D, T], BF16, kind="ExternalInput").ap()
    xkT = nc.dram_tensor("xkT", [D, T], BF16, kind="ExternalInput").ap()
    xvT = nc.dram_tensor("xvT", [D, T], BF16, kind="ExternalInput").ap()
    Wq = nc.dram_tensor("Wq", [D, D], BF16, kind="ExternalInput").ap()
    Wk = nc.dram_tensor("Wk", [D, D], BF16, kind="ExternalInput").ap()
    Wv = nc.dram_tensor("Wv", [D, D], BF16, kind="ExternalInput").ap()
    Wo = nc.dram_tensor("Wo", [D, D], BF16, kind="ExternalInput").ap()
    bq = nc.dram_tensor("bq", [D], F32, kind="ExternalInput").ap()
    bk = nc.dram_tensor("bk", [D], F32, kind="ExternalInput").ap()
    bv = nc.dram_tensor("bv", [D], F32, kind="ExternalInput").ap()
    bo = nc.dram_tensor("bo", [D], F32, kind="ExternalInput").ap()
    w1 = nc.dram_tensor("sf_w1", [L, FF], BF16, kind="ExternalInput").ap()
    b1 = nc.dram_tensor("sf_b1", [FF], F32, kind="ExternalInput").ap()
    w2 = nc.dram_tensor("sf_w2", [FF, L], BF16, kind="ExternalInput").ap()
    b2 = nc.dram_tensor("sf_b2", [L], F32, kind="ExternalInput").ap()
    out_d = nc.dram_tensor("out", [T, D], BF16, kind="ExternalOutput").ap()

    with tile.TileContext(nc) as tc, contextlib.ExitStack() as ctx:
        singles = ctx.enter_context(tc.tile_pool(name="singles", bufs=1))
        p_xt = ctx.enter_context(tc.tile_pool(name="xt", bufs=2))
        p_qk = ctx.enter_context(tc.tile_pool(name="qk", bufs=2))
        p_v = ctx.enter_context(tc.tile_pool(name="v", bufs=4))
        p_ssb = ctx.enter_context(tc.tile_pool(name="ssb", bufs=3))
        p_esb = ctx.enter_context(tc.tile_pool(name="esb", bufs=3))
        p_th = ctx.enter_context(tc.tile_pool(name="th", bufs=2))
        p_gel = ctx.enter_context(tc.tile_pool(name="gel", bufs=3))
        p_asc = ctx.enter_context(tc.tile_pool(name="asc", bufs=3))
        p_osb = ctx.enter_context(tc.tile_pool(name="osb", bufs=3))
        p_small = ctx.enter_context(tc.tile_pool(name="small", bufs=4))
        # PSUM: 8 banks. sc: 2x2 banks (scores/h2 then attention out),
        # h1: 2x1 bank (the two 284-wide h1 halves), pp: 2x1 bank.
        ps_sc = ctx.enter_context(tc.tile_pool(name="sc", bufs=2, space="PSUM"))
        ps_h1 = ctx.enter_context(tc.tile_pool(name="h1", bufs=2, space="PSUM"))
        ps_pp = ctx.enter_context(tc.tile_pool(name="pp", bufs=2, space="PSUM"))

        # ---- weights / constants.  q/k weights + group-0 activations are
        # DMA'd first so the PE can start projections asap. ----
        def w_tiles(w_ap, name):
            t = singles.tile([128, 4, D], BF16, tag=f"w_{name}")
            nc.gpsimd.dma_start(out=t, in_=w_ap.rearrange("(j p) d -> p j d", p=128))
            return t

        def b_tile(b_ap, name):
            t = singles.tile([128, 4], F32, tag=f"b_{name}")
            nc.gpsimd.dma_start(out=t, in_=b_ap.rearrange("(j p) -> p j", p=128))
            return t

        Wq_sb = w_tiles(Wq, "q")
        Wk_sb = w_tiles(Wk, "k")
        bq_sb = b_tile(bq, "q")
        bk_sb = b_tile(bk, "k")

        xq3 = xqT.rearrange("(j p) t -> p j t", p=128)
        xk3 = xkT.rearrange("(j p) t -> p j t", p=128)
        xv3 = xvT.rearrange("(j p) t -> p j t", p=128)

        xt_tiles = {}

        def issue_xt(g):
            t0 = g * GT
            xtq = p_xt.tile([128, 4, GT], BF16, tag="xtq")
            xtk = p_xt.tile([128, 4, GT], BF16, tag="xtk")
            xtv = p_xt.tile([128, 4, GT], BF16, tag="xtv")
            nc.gpsimd.dma_start(out=xtq, in_=xq3[:, :, t0:t0 + GT])
            nc.gpsimd.dma_start(out=xtk, in_=xk3[:, :, t0:t0 + GT])
            nc.gpsimd.dma_start(out=xtv, in_=xv3[:, :, t0:t0 + GT])
            xt_tiles[g] = (xtq, xtk, xtv)

        issue_xt(0)

        Wv_sb = w_tiles(Wv, "v")
        Wo_sb = w_tiles(Wo, "o")

        bq8_sb = singles.tile([128, 4], F32, tag="bq8")
        nc.scalar.mul(bq8_sb, bq_sb, 0.125)

        # w1 with b1 folded in as stationary row 71
        w1b_sb = singles.tile([72, FF], BF16, tag="w1b")
        nc.gpsimd.dma_start(out=w1b_sb[0:L, :], in_=w1)
        nc.gpsimd.dma_start(out=w1b_sb[L:L + 1, :],
                            in_=b1.rearrange("(o f) -> o f", o=1))
        # w2 pre-halved (gelu chain produces 2*gelu)
        w2_sb = singles.tile([FF, L], BF16, tag="w2")
        nc.gpsimd.dma_start(out=w2_sb, in_=w2)
        w2h_sb = singles.tile([FF, L], BF16, tag="w2h")
        nc.scalar.mul(w2h_sb, w2_sb, 0.5)
        b2_sb = singles.tile([L, 1], F32, tag="b2")
        nc.gpsimd.dma_start(out=b2_sb, in_=b2.rearrange("(p o) -> p o", o=1))

        ident = singles.tile([L, L], BF16, tag="ident")

        def emit_consts():
            # deferred so the group-0 projections lead the PE queue:
            # nothing here is needed before iteration ~2
            make_identity(nc, ident)
            # bo_eff = bo + bv @ Wo, replicated to [128, D]
            bv_sb = singles.tile([128, 4], BF16, tag="bv")
            nc.gpsimd.dma_start(out=bv_sb,
                                in_=bv.rearrange("(j p) -> p j", p=128))
            bo_sb = singles.tile([1, D], F32, tag="bo")
            nc.gpsimd.dma_start(out=bo_sb,
                                in_=bo.rearrange("(o d) -> o d", o=1))
            ps_bvwo = ps_pp.tile([1, D], F32, tag="pp")
            for j in range(4):
                nc.tensor.matmul(ps_bvwo, bv_sb[:, j:j + 1], Wo_sb[:, j, :],
                                 start=(j == 0), stop=(j == 3))
            boeff_row = singles.tile([1, D], F32, tag="boeffrow")
            nc.vector.tensor_add(boeff_row, ps_bvwo, bo_sb)
            ones_f32 = singles.tile([1, 128], F32, tag="ones1")
            nc.vector.memset(ones_f32, 1.0)
            ps_rep = ps_pp.tile([128, D], F32, tag="pp")
            nc.tensor.matmul(ps_rep, ones_f32, boeff_row, start=True,
                             stop=True)
            nc.vector.tensor_copy(out=BO_sb, in_=ps_rep)

        BO_sb = singles.tile([128, D], F32, tag="BO")
        attnT = singles.tile([128, 4, T], BF16, tag="attnT")

        # ---- filler machinery: each unit emits ~0.5-0.9us of PE work
        # whose inputs are already available, used to bridge the PE over
        # the scalar/vector latency chains of the sf-net.  pq holds the
        # next group's q/k projection units (deadline: that group's
        # scores); fq holds output-projection units (no deadline). ----
        pq = deque()
        fq = deque()

        def fill(n):
            for _ in range(n):
                if pq:
                    pq.popleft()()
                elif fq:
                    fq.popleft()()
                else:
                    break

        qk_tiles = {}

        def ensure_qk(g):
            if g not in qk_tiles:
                qT = p_qk.tile([128, 4, GT], BF16, tag="qT")
                kT = p_qk.tile([128, 4, GT], BF16, tag="kT")
                qk_tiles[g] = (qT, kT)
            return qk_tiles[g]

        def make_proj_unit(g, which, dt_, hf):
            def emit():
                qT, kT = ensure_qk(g)
                xtq, xtk, _ = xt_tiles[g]
                c0 = hf * HALF
                pq = ps_pp.tile([128, HALF], F32, tag="pp")
                if which == "q":
                    for j in range(4):
                        nc.tensor.matmul(
                            pq, Wq_sb[:, j, dt_ * 128:(dt_ + 1) * 128],
                            xtq[:, j, c0:c0 + HALF],
                            start=(j == 0), stop=(j == 3))
                    nc.scalar.activation(
                        out=qT[:, dt_, c0:c0 + HALF], in_=pq, func=AF.Identity,
                        bias=bq8_sb[:, dt_:dt_ + 1], scale=0.125)
                else:
                    for j in range(4):
                        nc.tensor.matmul(
                            pq, Wk_sb[:, j, dt_ * 128:(dt_ + 1) * 128],
                            xtk[:, j, c0:c0 + HALF],
                            start=(j == 0), stop=(j == 3))
                    nc.vector.tensor_scalar_add(
                        out=kT[:, dt_, c0:c0 + HALF], in0=pq,
                        scalar1=bk_sb[:, dt_:dt_ + 1])
            return emit

        def make_outproj_unit(off, w):
            def emit():
                po = ps_pp.tile([128, D], F32, tag="pp")
                for j in range(4):
                    nc.tensor.matmul(po[0:w], attnT[:, j, off:off + w],
                                     Wo_sb[:, j, :],
                                     start=(j == 0), stop=(j == 3))
                osb = p_osb.tile([128, D], BF16, tag="osb")
                nc.vector.tensor_add(osb[0:w], po[0:w], BO_sb[0:w])
                nc.sync.dma_start(out=out_d[off:off + w, :], in_=osb[0:w])
            return emit

        def outproj_units(t0, t1):
            units = []
            off = t0
            while off < t1:
                w = min(128, t1 - off)
                units.append(make_outproj_unit(off, w))
                off += w
            return units

        # ---- per-batch pipeline stages (n = global batch 0..63) ----
        st = {}      # n -> (S_ps, S3, Ssb)
        Ef = {}      # n -> E flat view
        vq = {}      # n -> v_sb
        ascd = {}    # n -> asc

        def V_stage(n):
            g, bl = n // GB, n % GB
            _, _, xtv = xt_tiles[g]
            pv = ps_pp.tile([L, D], F32, tag="pp")
            for j in range(4):
                nc.tensor.matmul(pv, xtv[:, j, bl * L:bl * L + L],
                                 Wv_sb[:, j, :],
                                 start=(j == 0), stop=(j == 3))
            v_sb = p_v.tile([L, H, DH + 1], BF16, tag="v")
            nc.gpsimd.memset(v_sb[:, :, DH:DH + 1], 1.0)
            nc.scalar.mul(v_sb[:, :, 0:DH],
                          pv.rearrange("p (h d) -> p h d", h=H), 1.0)
            vq[n] = v_sb

        def S_stage(n):
            # scores S' = [k, q], parity-grouped heads (stationary
            # partition base must not flip 0<->64 inside a bank group)
            g, bl = n // GB, n % GB
            qT, kT = ensure_qk(g)
            tc0 = bl * L
            S_ps = ps_sc.tile([L, 1024], F32, tag="sc")
            S3 = S_ps.rearrange("p (b r) -> p b r", b=2)[:, :, 0:4 * L]
            Ssb = p_ssb.tile([72, 2, 4 * L], BF16, tag="Ssb")
            # engines need 32-aligned start partitions: memset rows
            # 64..71 to 1.0, then the casts overwrite rows 64..70 with
            # scores, leaving the ones-row (for the b1 fold) at 71.
            nc.gpsimd.memset(Ssb[64:72, :, :], 1.0)
            for i in range(2):
                for hh in range(4):  # head 2*hh+i
                    off = 512 * i + L * hh
                    nc.tensor.matmul(
                        S_ps[:, off:off + L],
                        kT[64 * i:64 * i + 64, hh, tc0:tc0 + L],
                        qT[64 * i:64 * i + 64, hh, tc0:tc0 + L],
                        start=(hh == 0), stop=(hh == 3))
            # bank0 cast on vector (h1a's input, fastest path), bank1
            # cast on scalar (balances the vector queue)
            nc.vector.tensor_copy(out=Ssb[0:L, 0, :], in_=S3[:, 0, :])
            nc.scalar.mul(Ssb[0:L, 1, :], S3[:, 1, :], 1.0)
            st[n] = (S_ps, S3, Ssb)

        def H1_stage(n):
            _, _, Ssb = st[n]
            h1h = []
            for hf in range(2):
                h1 = ps_h1.tile([FF, HALF], F32, tag="h1")
                nc.tensor.matmul(h1, w1b_sb, Ssb[:, hf, :],
                                 start=True, stop=True)
                h1h.append(h1)
            gels = []
            for hf in range(2):
                h1 = h1h[hf]
                # sigmoid-approx gelu: x*sigmoid(1.702x) =
                # 0.5x(1+tanh(0.851x)) -- tanh lives in the exp LUT set,
                # no cubic needed.  Tanh on scalar straight from PSUM,
                # one fused (th+1)*x step on vector; w2 pre-halved.
                th = p_th.tile([FF, HALF], BF16, tag="th")
                nc.scalar.activation(out=th, in_=h1, func=AF.Tanh,
                                     scale=0.851)
                gel = p_gel.tile([FF, HALF], BF16, tag="gel")
                nc.vector.scalar_tensor_tensor(
                    out=gel, in0=th, scalar=1.0, in1=h1,
                    op0=ALU.add, op1=ALU.mult)
                gels.append(gel)
            st[n] = st[n] + (gels,)

        def H2_stage(n):
            S_ps, S3, Ssb, gels = st[n]
            for hf in range(2):
                nc.tensor.matmul(
                    S_ps[:, 512 * hf:512 * hf + 4 * L], w2h_sb, gels[hf],
                    start=False, stop=True, skip_group_check=True)
            # softmax numerator (no max subtraction), per bank so the
            # attention for heads of bank0 can start early
            E_sb = p_esb.tile([L, 2, 4 * L], BF16, tag="E")
            for hf in range(2):
                nc.scalar.activation(out=E_sb[:, hf, :], in_=S3[:, hf, :],
                                     func=AF.Exp, bias=b2_sb, scale=1.0)
            Ef[n] = E_sb.rearrange("p b r -> p (b r)")

        def A_stage(n):
            # attention + denominators; E/pa column block p hosts head
            # h = 2*(p%4) + p//4.
            Eflat = Ef.pop(n)
            v_sb = vq.pop(n)
            pa = ps_sc.tile([L, 1024], F32, tag="sc")
            for p in range(H):
                h = 2 * (p % 4) + (p // 4)
                off = 512 * (p // 4) + (DH + 1) * (p % 4)
                nc.tensor.matmul(
                    pa[:, off:off + DH + 1],
                    Eflat[:, L * p:L * p + L], v_sb[:, h, :],
                    start=(p % 4 == 0), stop=(p % 4 == 3))
            recip = p_small.tile([L, 2, 4], F32, tag="recip")
            nc.vector.reciprocal(
                out=recip,
                in_=bass.AP(tensor=pa.tensor, offset=pa.offset + DH,
                            ap=[pa.ap[0], [512, 2], [DH + 1, 4]]))
            # scale + cast in one strided op; bank b block hh lands at
            # col 128*hh + 64*b
            asc = p_asc.tile([L, D], BF16, tag="asc")
            nc.vector.tensor_mul(
                bass.AP(tensor=asc.tensor, offset=asc.offset,
                        ap=[asc.ap[0], [DH, 2], [2 * DH, 4], [1, DH]]),
                bass.AP(tensor=pa.tensor, offset=pa.offset,
                        ap=[pa.ap[0], [512, 2], [DH + 1, 4], [1, DH]]),
                bass.AP(tensor=recip.tensor, offset=recip.offset,
                        ap=[recip.ap[0], [4, 2], [1, 4], [0, DH]]))
            ascd[n] = asc

        def Tr_stage(n):
            # transpose attn rows to [feat, tok] and assemble
            asc = ascd.pop(n)
            del st[n]
            tp = ps_pp.tile([128, 4, L + 1], BF16, tag="pp")
            for j in range(4):
                nc.tensor.transpose(tp[:, j, 0:L],
                                    asc[:, 128 * j:128 * (j + 1)], ident)
            nc.vector.tensor_copy(out=attnT[:, :, n * L:(n + 1) * L],
                                  in_=tp[:, :, 0:L])

        def group_boundary(n):
            g = n // GB
            if n % GB == 0:
                if g + 1 < G:
                    issue_xt(g + 1)
                # group g's projections MUST be fully emitted before its
                # scores read qT/kT (emission order defines the deps)
                while pq:
                    pq.popleft()()
                if g + 1 < G:
                    for dt_ in range(4):
                        for hf in range(2):
                            pq.append(make_proj_unit(g + 1, "q", dt_, hf))
                            pq.append(make_proj_unit(g + 1, "k", dt_, hf))
            if n % GB == 2:
                # output-projection pushes: the covered tokens were
                # written by Tr(n-3) emitted in the previous iteration
                if g >= 2 and g % 2 == 0:
                    d = g // 2 - 1
                    fq.extend(outproj_units(d * DUO, (d + 1) * DUO))
                if g == G - 1:
                    fq.extend(outproj_units((G - 2) * GT, (G - 1) * GT))

        # ---- flat skewed software pipeline over all 64 batches ----
        for dt_ in range(4):
            for hf in range(2):
                make_proj_unit(0, "q", dt_, hf)()
                make_proj_unit(0, "k", dt_, hf)()
        group_boundary(0)
        V_stage(0)
        V_stage(1)
        S_stage(0)
        H1_stage(0)
        emit_consts()
        fill(1)
        for n in range(1, BC):
            group_boundary(n)
            H2_stage(n - 1)
            if n + 1 < BC:
                V_stage(n + 1)
            S_stage(n)
            A_stage(n - 1)
            fill(1)
            H1_stage(n)
            fill(1)
            if n >= 2:
                # transpose skewed one extra iteration so the normalize
                # multiply has a full iteration of slack on vector
                Tr_stage(n - 2)
            fill(1)
        H2_stage(BC - 1)
        A_stage(BC - 1)
        Tr_stage(BC - 2)
        Tr_stage(BC - 1)
        while pq:
            pq.popleft()()
        while fq:
            fq.popleft()()
        for u in outproj_units((G - 1) * GT, T):
            u()

    nc.compile()
    return nc


def _get_nc():
    if "nc" not in _CACHE:
        _CACHE["nc"] = _build()
    return _CACHE["nc"]


def _prep_in_maps(inputs):
    import ml_dtypes
    bf16 = ml_dtypes.bfloat16
    f32 = lambda a: np.ascontiguousarray(np.asarray(a, dtype=np.float32))
    shared = {k: f32(inputs[k]) for k in ("bq", "bk", "bv", "bo",
                                          "sf_b1", "sf_b2")}
    for k in ("Wq", "Wk", "Wv", "Wo", "sf_w1", "sf_w2"):
        shared[k] = np.ascontiguousarray(
            np.asarray(inputs[k], dtype=np.float32).astype(bf16))
    xT = {}
    for key, name in (("query", "xqT"), ("key", "xkT"), ("value", "xvT")):
        # [B, L, D] -> [D, B, L], feature-major + bf16 (the device DMA
        # was down-converting to bf16 anyway; halves HBM traffic)
        xT[name] = np.asarray(inputs[key], dtype=np.float32).astype(
            bf16).transpose(2, 0, 1)
    in_maps = []
    for c in range(N_CORES):
        m = dict(shared)
        for name in ("xqT", "xkT", "xvT"):
            m[name] = np.ascontiguousarray(
                xT[name][:, c * BC:(c + 1) * BC, :]).reshape(D, T)
        in_maps.append(m)
    return in_maps


def run(inputs, trace=False):
    nc = _get_nc()
    in_maps = _prep_in_maps(inputs)
    res = bass_utils.run_bass_kernel_spmd(
        nc, in_maps, core_ids=list(range(N_CORES)), trace=trace)
    out = np.concatenate(
        [np.asarray(res.results[c]["out"]).astype(np.float32).reshape(
            BC, L, D) for c in range(N_CORES)],
        axis=0)
    return out, res


def kernel(**inputs) -> np.ndarray:
    out, _ = run(inputs, trace=False)
    return out
